# revision 16
# baseline (speedup 1.0000x reference)
"""Trainium2 Bass kernel for nn_MultiHeadedAttentionWithGate.

Math (per molecule, validated against reference):
  The reference's reshapes are all flat views, so with u = "virtual row"
  (1024 per molecule), the computation is per-u over contiguous flat
  segments: K/V/M rows of 320 (10 nei x 32), X rows of 640 (10 x 64),
  q rows of 32.

Layout trick ("phase decomposition"): u = 4*g + r.  For fixed phase
r (0..3) and g on partitions, every tensor's u-row is a contiguous DRAM
segment (partition stride 2560 elems for X), and the projections
K/V/M[u-layout] decompose into matmuls over X^T chunks whose row sets
are stride-5 (rows 5g+d, d in 0..4) -- an affine AP.  The 20 (d, f-chunk)
X^T chunks per 128-g tile are exactly the PE transposes of the 4 phases'
Xu tiles chunked by 128 columns.  All softmax/max/mean reductions are
then per-partition (free-axis) ops.

Sharding: data-parallel over batch: 8 molecules per core x 8 cores.
"""

import sys

for _p in ("/opt/trn_rl_repo", "/root/.axon_site/_ro/trn_rl_repo"):
    if _p not in sys.path:
        sys.path.insert(0, _p)

from contextlib import ExitStack

import numpy as np

import concourse.bass as bass
import concourse.mybir as mybir
from concourse import bacc
from concourse.tile import TileContext

F16 = mybir.dt.float16
F32 = mybir.dt.float32
EXP = mybir.ActivationFunctionType.Exp
ADD = mybir.AluOpType.add
MAX = mybir.AluOpType.max
MULT = mybir.AluOpType.mult
AXL_X = mybir.AxisListType.X

N_CORES = 8
BM = 8          # molecules per core
A = 128         # atoms
NEI = 10
D = 256
D2 = 512


def build_nc(with_bias: bool, bg_val: float) -> bass.Bass:
    nc = bacc.Bacc("TRN2", target_bir_lowering=False)

    x_h = nc.declare_dram_parameter("x", [BM, A * NEI, D2], F32, isOutput=False)
    qin_h = nc.declare_dram_parameter("qin", [BM, A, D], F32, isOutput=False)
    wcat_h = nc.declare_dram_parameter("wcat", [128, 4, 768], F16, isOutput=False)
    wq_h = nc.declare_dram_parameter("wq", [128, 2, 256], F16, isOutput=False)
    ident_h = nc.declare_dram_parameter("ident", [128, 128], F16, isOutput=False)
    ssel_h = nc.declare_dram_parameter("ssel", [128, 32], F16, isOutput=False)
    s2sel_h = nc.declare_dram_parameter("s2sel", [32, 128], F16, isOutput=False)
    wgc_h = nc.declare_dram_parameter("wg_cur", [128, 32], F32, isOutput=False)
    wge_h = nc.declare_dram_parameter("wg_emax", [128, 32], F32, isOutput=False)
    wga_h = nc.declare_dram_parameter("wg_ave", [128, 640], F32, isOutput=False)
    if with_bias:
        bcat_h = nc.declare_dram_parameter("bcat", [1, 3, 256], F16, isOutput=False)
        bq_h = nc.declare_dram_parameter("bq", [1, 256], F16, isOutput=False)
        ones_h = nc.declare_dram_parameter("ones", [1, 128], F16, isOutput=False)
    out_h = nc.declare_dram_parameter("out", [BM, A, D], F32, isOutput=True)

    # flat per-molecule views: u = 4g + r = 512*G + 4*p + r
    x5 = (x_h[:].rearrange("b n c -> b (n c)")
          .rearrange("b (g p r t) -> b g r p t", g=2, p=128, r=4, t=640))
    q5 = (qin_h[:].rearrange("b a c -> b (a c)")
          .rearrange("b (g p r k) -> b g r p k", g=2, p=128, r=4, k=32))
    o5 = (out_h[:].rearrange("b a c -> b (a c)")
          .rearrange("b (g p r k) -> b g r p k", g=2, p=128, r=4, k=32))

    with TileContext(nc) as tc, ExitStack() as ctx:
        consts = ctx.enter_context(tc.tile_pool(name="consts", bufs=1))
        sb_xu = ctx.enter_context(tc.tile_pool(name="xu", bufs=10))
        sb_x16 = ctx.enter_context(tc.tile_pool(name="x16", bufs=3))
        sb_xt = ctx.enter_context(tc.tile_pool(name="xt", bufs=44))
        sb_big = ctx.enter_context(tc.tile_pool(name="big", bufs=3))
        sb_ew = ctx.enter_context(tc.tile_pool(name="ew", bufs=4))
        sb_stash = ctx.enter_context(tc.tile_pool(name="stash", bufs=18))
        sb_q = ctx.enter_context(tc.tile_pool(name="qp", bufs=2))
        ps_proj = ctx.enter_context(tc.tile_pool(name="pp", bufs=6, space="PSUM"))
        ps_misc = ctx.enter_context(tc.tile_pool(name="pm", bufs=2, space="PSUM"))
        dram = ctx.enter_context(tc.tile_pool(name="dram", bufs=1, space="DRAM"))

        def cload(h, shape, dtype):
            t = consts.tile(shape, dtype, tag=h.name)
            nc.sync.dma_start(out=t, in_=h[:])
            return t

        wcat_t = cload(wcat_h, [128, 4, 768], F16)
        wq_t = cload(wq_h, [128, 2, 256], F16)
        ident_t = cload(ident_h, [128, 128], F16)
        ssel_t = cload(ssel_h, [128, 32], F16)
        s2sel_t = cload(s2sel_h, [32, 128], F16)
        wgc_t = cload(wgc_h, [128, 32], F32)
        wge_t = cload(wge_h, [128, 32], F32)
        wga_t = cload(wga_h, [128, 640], F32)
        if with_bias:
            bcat_t = cload(bcat_h, [1, 3, 256], F16)
            bq_t = cload(bq_h, [1, 256], F16)
            ones_t = cload(ones_h, [1, 128], F16)

        qdram = dram.tile([BM, A * D], F32)

        for mol in range(BM):
            # ---- q projection (natural layout) -> DRAM scratch ----
            qin_t = sb_q.tile([128, 256], F32, tag="qin")
            nc.sync.dma_start(out=qin_t, in_=qin_h[mol])
            qin16 = sb_q.tile([128, 256], F16, tag="qin16")
            nc.scalar.copy(out=qin16, in_=qin_t)
            qT = []
            for w in range(2):
                tp = ps_misc.tile([128, 128], F16, tag="pm")
                nc.tensor.transpose(tp, qin16[:, 128 * w:128 * (w + 1)], ident_t)
                xt = sb_q.tile([128, 128], F16, tag="qT")
                nc.vector.tensor_copy(out=xt, in_=tp)
                qT.append(xt)
            qpsum = ps_misc.tile([128, 256], F32, tag="pm")
            nc.tensor.matmul(qpsum, qT[0], wq_t[:, 0, :], start=True, stop=False)
            nc.tensor.matmul(qpsum, qT[1], wq_t[:, 1, :],
                             start=False, stop=not with_bias)
            if with_bias:
                nc.tensor.matmul(qpsum, ones_t, bq_t, start=False, stop=True)
            qnat = sb_q.tile([128, 256], F32, tag="qnat")
            nc.vector.tensor_copy(out=qnat, in_=qpsum)
            nc.sync.dma_start(
                out=qdram[mol].rearrange("(a c) -> a c", a=128), in_=qnat)
            q_read = qdram[mol].rearrange(
                "(g p r k) -> g r p k", g=2, p=128, r=4, k=32)

            stash0 = {}
            for G in range(2):
                # ---- load Xu (4 phases), cast to f16, transpose into XT ----
                xu = []
                XT = {}
                for r in range(4):
                    xt_in = sb_xu.tile([128, 640], F32, tag="xu")
                    nc.sync.dma_start(out=xt_in, in_=x5[mol, G, r])
                    xu.append(xt_in)
                for r in range(4):
                    x16 = sb_x16.tile([128, 640], F16, tag="x16")
                    nc.scalar.copy(out=x16, in_=xu[r])
                    for w in range(5):
                        d, fc = divmod(5 * r + w, 4)
                        tp = ps_misc.tile([128, 128], F16, tag="pm")
                        nc.tensor.transpose(
                            tp, x16[:, 128 * w:128 * (w + 1)], ident_t)
                        xt = sb_xt.tile([128, 128], F16, tag="xt")
                        nc.vector.tensor_copy(out=xt, in_=tp)
                        XT[(d, fc)] = xt

                for r in range(4):
                    # ---- projections K|V|M in u-layout [128, 320] ----
                    wA = 256 - 64 * r
                    ranges = [(r, 0, wA, 64 * r), (r + 1, wA, 320 - wA, 0)]
                    pk = ps_proj.tile([128, 320], F32, tag="pp")
                    pv = ps_proj.tile([128, 320], F32, tag="pp")
                    pm = ps_proj.tile([128, 320], F32, tag="pp")
                    for (d, t0, wd, e0) in ranges:
                        for fc in range(4):
                            st = fc == 0
                            sp = (fc == 3) and not with_bias
                            for i, pt in enumerate((pk, pv, pm)):
                                nc.tensor.matmul(
                                    pt[:, t0:t0 + wd], XT[(d, fc)],
                                    wcat_t[:, fc, 256 * i + e0:256 * i + e0 + wd],
                                    start=st, stop=sp)
                        if with_bias:
                            for i, pt in enumerate((pk, pv, pm)):
                                nc.tensor.matmul(
                                    pt[:, t0:t0 + wd], ones_t,
                                    bcat_t[:, i, e0:e0 + wd],
                                    start=False, stop=True)

                    # ---- elementwise stage (all per-partition) ----
                    cur = sb_ew.tile([128, 32], F32, tag="cur")
                    nc.sync.dma_start(out=cur, in_=q5[mol, G, r])
                    qu = sb_ew.tile([128, 32], F32, tag="qu")
                    nc.sync.dma_start(out=qu, in_=q_read[G, r])

                    smul = sb_big.tile([128, 320], F32, tag="smul")
                    nc.vector.tensor_mul(
                        smul, pk, qu.unsqueeze(1).broadcast_to([128, 10, 32]))
                    score = sb_ew.tile([128, 10], F32, tag="score")
                    nc.vector.tensor_reduce(
                        out=score, in_=smul.rearrange("p (j k) -> p j k", j=10),
                        axis=AXL_X, op=ADD)
                    ex = sb_ew.tile([128, 10], F16, tag="ex")
                    aden = sb_ew.tile([128, 1], F32, tag="aden")
                    nc.scalar.activation(out=ex, in_=score, func=EXP,
                                         accum_out=aden)
                    amul = sb_big.tile([128, 320], F32, tag="amul")
                    nc.vector.tensor_mul(
                        amul, pv, ex.unsqueeze(2).broadcast_to([128, 10, 32]))
                    araw = sb_stash.tile([128, 32], F32, tag="araw")
                    nc.vector.tensor_reduce(
                        out=araw, in_=amul.rearrange("p (j k) -> p k j", j=10),
                        axis=AXL_X, op=ADD)
                    emax = sb_ew.tile([128, 32], F32, tag="emax")
                    nc.vector.tensor_reduce(
                        out=emax, in_=pm.rearrange("p (j k) -> p k j", j=10),
                        axis=AXL_X, op=MAX)
                    ra = sb_stash.tile([128, 1], F32, tag="ra")
                    nc.vector.reciprocal(out=ra, in_=aden)

                    # gate logit pieces: gpsimd mul (SBUF-only), DVE reduce
                    pr1 = sb_ew.tile([128, 32], F32, tag="pr1")
                    nc.gpsimd.tensor_mul(pr1, cur, wgc_t)
                    gcur = sb_ew.tile([128, 1], F32, tag="gcur")
                    nc.vector.tensor_reduce(out=gcur, in_=pr1,
                                            axis=AXL_X, op=ADD)
                    pr2 = sb_ew.tile([128, 32], F32, tag="pr2")
                    nc.gpsimd.tensor_mul(pr2, emax, wge_t)
                    gemx = sb_ew.tile([128, 1], F32, tag="gemx")
                    nc.vector.tensor_reduce(out=gemx, in_=pr2,
                                            axis=AXL_X, op=ADD)
                    pr3 = sb_big.tile([128, 640], F32, tag="pr3")
                    nc.gpsimd.tensor_mul(pr3, xu[r], wga_t)
                    gave = sb_ew.tile([128, 1], F32, tag="gave")
                    nc.vector.tensor_reduce(out=gave, in_=pr3,
                                            axis=AXL_X, op=ADD)
                    gl1 = sb_ew.tile([128, 1], F32, tag="gl1")
                    nc.gpsimd.tensor_add(gl1, gcur, gemx)
                    gl2 = sb_ew.tile([128, 1], F32, tag="gl2")
                    nc.gpsimd.tensor_add(gl2, gl1, gave)
                    eg32 = sb_stash.tile([128, 1], F32, tag="eg32")
                    nc.scalar.activation(out=eg32, in_=gl2, func=EXP,
                                         bias=float(bg_val))
                    eg = sb_stash.tile([128, 1], F16, tag="eg")
                    nc.vector.tensor_copy(out=eg, in_=eg32)

                    if G == 0:
                        stash0[r] = (araw, ra, eg, eg32)
                    else:
                        # ---- gate softmax over heads + output ----
                        gd = ps_misc.tile([32, 1], F32, tag="pm")
                        nc.tensor.matmul(gd, ssel_t, stash0[r][2],
                                         start=True, stop=False)
                        nc.tensor.matmul(gd, ssel_t, eg, start=False, stop=True)
                        rg = sb_ew.tile([32, 1], F32, tag="rg")
                        nc.vector.reciprocal(out=rg, in_=gd)
                        rg16 = sb_ew.tile([32, 1], F16, tag="rg16")
                        nc.vector.tensor_copy(out=rg16, in_=rg)
                        inv = ps_misc.tile([128, 1], F32, tag="pm")
                        nc.tensor.matmul(inv, s2sel_t, rg16,
                                         start=True, stop=True)
                        for gg, (araw_g, ra_g, eg_g, eg32_g) in (
                                (0, stash0[r]), (1, (araw, ra, eg, eg32))):
                            c2 = sb_ew.tile([128, 1], F32, tag="c2")
                            nc.vector.tensor_scalar(
                                out=c2, in0=inv, scalar1=ra_g, scalar2=eg32_g,
                                op0=MULT, op1=MULT)
                            outt = sb_ew.tile([128, 32], F32, tag="outt")
                            nc.gpsimd.tensor_scalar_mul(outt, araw_g, c2)
                            nc.sync.dma_start(out=o5[mol, gg, r], in_=outt)
    nc.finalize()
    return nc


def _prep_consts(Wq, bq, Wk, bk, Wv, bv, Wam, bam, Wg, bg):
    wcat = np.empty((128, 4, 768), np.float16)
    for i, W in enumerate((Wk, Wv, Wam)):
        for fc in range(4):
            wcat[:, fc, 256 * i:256 * (i + 1)] = W[128 * fc:128 * (fc + 1), :]
    wq = np.empty((128, 2, 256), np.float16)
    for fc in range(2):
        wq[:, fc, :] = Wq[128 * fc:128 * (fc + 1), :]
    ident = np.eye(128, dtype=np.float16)
    p = np.arange(128)
    ssel = (p[:, None] % 32 == np.arange(32)[None, :]).astype(np.float16)
    s2sel = ssel.T.copy()
    wg = np.asarray(Wg[:, 0], np.float32)
    consts = {
        "wcat": wcat, "wq": wq, "ident": ident,
        "ssel": ssel, "s2sel": s2sel,
        "wg_cur": np.tile(wg[0:32], (128, 1)).astype(np.float32),
        "wg_emax": np.tile(wg[32:64], (128, 1)).astype(np.float32),
        "wg_ave": np.tile(np.tile(wg[64:128], NEI) * (1.0 / NEI),
                          (128, 1)).astype(np.float32),
    }
    with_bias = any(np.any(np.asarray(b) != 0) for b in (bq, bk, bv, bam))
    if with_bias:
        bcat = np.stack([np.asarray(bk), np.asarray(bv), np.asarray(bam)]
                        ).astype(np.float16)[None, :, :].reshape(1, 3, 256)
        consts["bcat"] = bcat
        consts["bq"] = np.asarray(bq, np.float16).reshape(1, 256)
        consts["ones"] = np.ones((1, 128), np.float16)
    return consts, with_bias, float(np.asarray(bg).reshape(-1)[0])


_CACHE = {}
TRACE = False       # set by test.py for profiling runs
LAST_RESULTS = None  # BassKernelResults from the most recent run


def kernel(input_multihead, input_q, Wq, bq, Wk, bk, Wv, bv, Wam, bam, Wg, bg):
    from concourse.bass_utils import run_bass_kernel_spmd

    consts, with_bias, bg_val = _prep_consts(
        Wq, bq, Wk, bk, Wv, bv, Wam, bam, Wg, bg)

    key = (with_bias, bg_val)
    if key not in _CACHE:
        _CACHE[key] = build_nc(with_bias, bg_val)
    nc = _CACHE[key]

    x = np.ascontiguousarray(np.asarray(input_multihead, np.float32))
    q = np.ascontiguousarray(np.asarray(input_q, np.float32))
    in_maps = []
    for c in range(N_CORES):
        m = {"x": x[BM * c:BM * (c + 1)], "qin": q[BM * c:BM * (c + 1)]}
        m.update(consts)
        in_maps.append(m)

    res = run_bass_kernel_spmd(nc, in_maps, list(range(N_CORES)), trace=TRACE)
    global LAST_RESULTS
    LAST_RESULTS = res
    return np.concatenate([res.results[c]["out"] for c in range(N_CORES)],
                          axis=0)
